# revision 10
# baseline (speedup 1.0000x reference)
"""DeepSeekV3-style MoE layer on 8 Trainium2 NeuronCores.

Sharding (expert-parallel, host-orchestrated dispatch):
  - Router (tiny) on host via jax-CPU, bit-exact with the reference.
  - Core e computes expert e over its routed tokens (gathered, transposed,
    zero-padded to capacity C) with wmean folded into the down projection.
  - Shared expert: 4-way token split x 2-way intermediate split.
  - Host combine: scatter-add routed outputs + shared outputs.

Device kernel (all matmul inputs fp16, f32 PSUM, fp16 staged outputs):
  Phase A (routed): token tiles [512, C-512].  The first tile runs the
  gate/up contraction k-outer across 8 parallel PSUM banks so the PE
  consumes k-chunks in DMA arrival order (minimal startup stall); the
  last two k-layers are m-staggered so silu/mul pipelining starts early.
  Phase B (shared): m-outer gate/up pairs.  Two phase-A down chunks and
  the tail-tile downs are deferred until after phase-B gate/up to cover
  the silu/mul latency ahead of phase-B downs.
"""

import os

os.environ.setdefault("JAX_PLATFORMS", "axon,cpu")

import numpy as np

# Problem constants (hardcoded per spec nn_DeepSeekV3MoE_11269994184873).
H = 1024       # hidden size
I = 512        # moe intermediate size
E = 8          # routed experts == n cores
K = 2          # experts per token
SI = 1024      # shared expert intermediate
B, S = 2, 1024
T = B * S      # 2048 tokens
P = 128
N_CORES = 8
TS = T // 4        # shared-expert tokens per core (512): 4-way token split
SIH = SI // 2      # shared-expert intermediate half per core: 2-way SI split
KH = H // P        # 8 k-chunks for H contraction
KI = I // P        # 4 k-chunks for I contraction
KS = SIH // P      # 4 k-chunks for SI-half contraction
MI = I // P        # 4 m-chunks of the routed intermediate
MH = H // P        # 8 m-chunks of hidden
MS = SIH // P      # 4 m-chunks of the shared intermediate half

_nc_cache: dict = {}
last_nc = None  # exposed for test harness (TimelineSim)


def _round_up(v, m):
    return ((v + m - 1) // m) * m


def _host_router(x, gate_w, lb_bias):
    """Replicate the reference router on CPU via jax (bit-exact scores/top-k)."""
    import jax
    import jax.numpy as jnp

    cpu = jax.devices("cpu")[0]
    with jax.default_device(cpu):
        xf = jnp.asarray(np.asarray(x, np.float32)).reshape(-1, H)
        logits = xf @ jnp.asarray(np.asarray(gate_w, np.float32)).T + jnp.asarray(
            np.asarray(lb_bias, np.float32)
        )
        scores = jax.nn.sigmoid(logits.astype(jnp.float32))
        topw, topi = jax.lax.top_k(scores, K)
        topw = (topw / (topw.sum(-1, keepdims=True) + 1e-8)).astype(jnp.float32)
        wmeans = []
        for e in range(E):
            m = topi == e
            cnt = m.sum()
            wmean = (topw * m).sum() / jnp.maximum(cnt, 1).astype(topw.dtype)
            wmeans.append(wmean)
        topi_np = np.asarray(topi)
        wmean_np = np.asarray(jnp.stack(wmeans), np.float32)
    return topi_np, wmean_np


def _build_bass(C):
    """Build the SPMD Bass program for capacity C (multiple of 16)."""
    from contextlib import ExitStack

    import concourse.bacc as bacc
    import concourse.mybir as mybir
    import concourse.tile as tile

    f32 = mybir.dt.float32
    f16 = mybir.dt.float16
    Silu = mybir.ActivationFunctionType.Silu

    nc = bacc.Bacc("TRN2", target_bir_lowering=False, debug=False,
                   num_devices=N_CORES)

    # DRAM I/O (per-core values, same shapes on every core), all fp16.
    xe = nc.dram_tensor("xe", [KH, P, C], f16, kind="ExternalInput")
    wg = nc.dram_tensor("wg", [KH, P, I], f16, kind="ExternalInput")
    wu = nc.dram_tensor("wu", [KH, P, I], f16, kind="ExternalInput")
    wd = nc.dram_tensor("wd", [KI, P, H], f16, kind="ExternalInput")
    xs = nc.dram_tensor("xs", [KH, P, TS], f16, kind="ExternalInput")
    sg = nc.dram_tensor("sg", [KH, P, SIH], f16, kind="ExternalInput")
    su = nc.dram_tensor("su", [KH, P, SIH], f16, kind="ExternalInput")
    sd = nc.dram_tensor("sd", [KS, P, H], f16, kind="ExternalInput")
    ye = nc.dram_tensor("ye", [MH, P, C], f16, kind="ExternalOutput")
    zs = nc.dram_tensor("zs", [MH, P, TS], f16, kind="ExternalOutput")

    # token tiles for the routed phase
    if C <= 512:
        a_tiles = [(0, C)]
    else:
        a_tiles = [(0, 512), (512, C - 512)]
    tn0 = a_tiles[0][1]

    with tile.TileContext(nc) as tc:
        with ExitStack() as ctx:
            const = ctx.enter_context(tc.tile_pool(name="const", bufs=1))
            tpool = ctx.enter_context(tc.tile_pool(name="tmp", bufs=2))
            psA = ctx.enter_context(tc.tile_pool(name="psA", bufs=4, space="PSUM"))
            psB = ctx.enter_context(tc.tile_pool(name="psB", bufs=4, space="PSUM"))

            # ---- SBUF tiles ----
            x_sb = const.tile([P, KH, C], f16, tag="x_sb")
            wg_sb = const.tile([P, KH, I], f16, tag="wg_sb")
            wu_sb = const.tile([P, KH, I], f16, tag="wu_sb")
            wd_sb = const.tile([P, KI, H], f16, tag="wd_sb")
            xs_sb = const.tile([P, KH, TS], f16, tag="xs_sb")
            sg_sb = const.tile([P, KH, SIH], f16, tag="sg_sb")
            su_sb = const.tile([P, KH, SIH], f16, tag="su_sb")
            sd_sb = const.tile([P, KS, H], f16, tag="sd_sb")
            h_a = const.tile([P, KI, C], f16, tag="h_a")
            h_s = const.tile([P, KS, TS], f16, tag="h_s")
            y_st = const.tile([P, MH, C], f16, tag="y_st")
            z_st = const.tile([P, MH, TS], f16, tag="z_st")

            # ---- input DMAs (SP queue, in arrival-order for the PE) ----
            def load_k(dst_sb, src, k0, k1):
                nc.sync.dma_start(
                    dst_sb[:, k0:k1, :],
                    src.ap()[k0:k1].rearrange("k p c -> p k c"),
                )

            for k0, k1 in [(0, 1), (1, 2), (2, 4), (4, 6), (6, 8)]:
                load_k(x_sb, xe, k0, k1)
                load_k(wg_sb, wg, k0, k1)
                load_k(wu_sb, wu, k0, k1)
            for k0, k1 in [(0, 2), (2, 4)]:
                load_k(wd_sb, wd, k0, k1)
            for k0, k1 in [(0, 4), (4, 8)]:
                load_k(xs_sb, xs, k0, k1)
            for k0, k1 in [(0, 4), (4, 8)]:
                load_k(sg_sb, sg, k0, k1)
            for k0, k1 in [(0, 4), (4, 8)]:
                load_k(su_sb, su, k0, k1)
            for k0, k1 in [(0, 2), (2, 4)]:
                load_k(sd_sb, sd, k0, k1)

            # ---- helpers ----
            _copy_flip = [0]

            def psum_copy(dst_ap, src_ap):
                # alternate Act / DVE for PSUM->fp16 staging copies
                if _copy_flip[0] & 1:
                    nc.scalar.copy(dst_ap, src_ap)
                else:
                    nc.vector.tensor_copy(dst_ap, src_ap)
                _copy_flip[0] += 1

            def silu_mul(pg, pu, h_tile, m, off, tn, name):
                tg = tpool.tile([P, 512], f32, tag="tg", name=f"tg{name}")
                nc.scalar.activation(tg[:, :tn], pg[:, :tn], Silu)
                nc.vector.tensor_mul(h_tile[:, m, off:off + tn], tg[:, :tn],
                                     pu[:, :tn])

            def gu_pair(w_g, w_u, x_t, h_tile, m, off, tn, nk, name):
                """m-outer gate/up pair for one m-chunk."""
                pg = psA.tile([P, 512], f32, tag="ps", name=f"pg{name}")
                for k in range(nk):
                    nc.tensor.matmul(pg[:, :tn], w_g[:, k, m * P:(m + 1) * P],
                                     x_t[:, k, off:off + tn],
                                     start=(k == 0), stop=(k == nk - 1))
                pu = psA.tile([P, 512], f32, tag="ps", name=f"pu{name}")
                for k in range(nk):
                    nc.tensor.matmul(pu[:, :tn], w_u[:, k, m * P:(m + 1) * P],
                                     x_t[:, k, off:off + tn],
                                     start=(k == 0), stop=(k == nk - 1))
                silu_mul(pg, pu, h_tile, m, off, tn, name)

            def down_chunk(w_d, h_tile, st_tile, m, off, tn, nk, name):
                py = psB.tile([P, 512], f32, tag="ps", name=f"py{name}")
                for k in range(nk):
                    nc.tensor.matmul(py[:, :tn], w_d[:, k, m * P:(m + 1) * P],
                                     h_tile[:, k, off:off + tn],
                                     start=(k == 0), stop=(k == nk - 1))
                psum_copy(st_tile[:, m, off:off + tn], py[:, :tn])

            # ---- phase A tile0 gate/up: k-outer startup ----
            # k layers 0..3 across 8 parallel psums (PE consumes in DMA
            # arrival order), then k4..7 per-m so completions stagger.
            pgs = [psA.tile([P, 512], f32, tag="ps", name=f"pg0_{m}")
                   for m in range(MI)]
            pus = [psB.tile([P, 512], f32, tag="ps", name=f"pu0_{m}")
                   for m in range(MI)]
            for k in range(4):
                for m in range(MI):
                    nc.tensor.matmul(pgs[m][:, :tn0],
                                     wg_sb[:, k, m * P:(m + 1) * P],
                                     x_sb[:, k, 0:tn0],
                                     start=(k == 0), stop=False)
                    nc.tensor.matmul(pus[m][:, :tn0],
                                     wu_sb[:, k, m * P:(m + 1) * P],
                                     x_sb[:, k, 0:tn0],
                                     start=(k == 0), stop=False)
            for m in range(MI):
                for k in (4, 5, 6, 7):
                    nc.tensor.matmul(pgs[m][:, :tn0],
                                     wg_sb[:, k, m * P:(m + 1) * P],
                                     x_sb[:, k, 0:tn0],
                                     start=False, stop=(k == 7))
                for k in (4, 5, 6, 7):
                    nc.tensor.matmul(pus[m][:, :tn0],
                                     wu_sb[:, k, m * P:(m + 1) * P],
                                     x_sb[:, k, 0:tn0],
                                     start=False, stop=(k == 7))
                silu_mul(pgs[m], pus[m], h_a, m, 0, tn0, f"t0_{m}")

            # ---- phase A tail-tile gate/up (m-outer) ----
            for off, tn in a_tiles[1:]:
                for m in range(MI):
                    gu_pair(wg_sb, wu_sb, x_sb, h_a, m, off, tn, KH,
                            f"t1_{m}")

            # ---- phase A downs for tile0, m0..m5 ----
            for m in range(6):
                down_chunk(wd_sb, h_a, y_st, m, 0, tn0, KI, f"dt0_{m}")

            # ---- phase B gate/up (m-outer) ----
            for m in range(MS):
                gu_pair(sg_sb, su_sb, xs_sb, h_s, m, 0, TS, KH, f"b_{m}")

            # ---- deferred phase A downs: tile0 m6/m7 + tail tiles ----
            for m in (6, 7):
                down_chunk(wd_sb, h_a, y_st, m, 0, tn0, KI, f"dt0_{m}")
            for off, tn in a_tiles[1:]:
                for m in range(MH):
                    down_chunk(wd_sb, h_a, y_st, m, off, tn, KI, f"dt1_{m}")
            # routed output DMA (Pool/SWDGE queue: no head-of-line blocking
            # of silu/copy work on Act)
            nc.gpsimd.dma_start(
                ye.ap().rearrange("m p c -> p m c"), y_st[:, :, :])

            # ---- phase B downs ----
            for m in range(MH):
                down_chunk(sd_sb, h_s, z_st, m, 0, TS, KS, f"db_{m}")
                if m % 2 == 1 and m < 5:
                    nc.gpsimd.dma_start(
                        zs.ap()[m - 1:m + 1].rearrange("m p c -> p m c"),
                        z_st[:, m - 1:m + 1, :])
            for m in (4, 5, 6, 7):
                nc.gpsimd.dma_start(
                    zs.ap()[m:m + 1].rearrange("m p c -> p m c"),
                    z_st[:, m:m + 1, :])

    nc.finalize()
    return nc


def _get_nc(C):
    global last_nc
    key = C
    if key not in _nc_cache:
        _nc_cache[key] = _build_bass(C)
    last_nc = _nc_cache[key]
    return _nc_cache[key]


def kernel(x, gate_w, lb_bias, expert_gate_w, expert_up_w, expert_down_w,
           shared_gate_w, shared_up_w, shared_down_w):
    from concourse.bass_utils import run_bass_kernel_spmd

    x = np.asarray(x, np.float32)
    gate_w = np.asarray(gate_w, np.float32)
    lb_bias = np.asarray(lb_bias, np.float32)
    egw = np.asarray(expert_gate_w, np.float32)
    euw = np.asarray(expert_up_w, np.float32)
    edw = np.asarray(expert_down_w, np.float32)
    sgw = np.asarray(shared_gate_w, np.float32)
    suw = np.asarray(shared_up_w, np.float32)
    sdw = np.asarray(shared_down_w, np.float32)

    xf = x.reshape(T, H)

    # ---- host router (replicates reference) ----
    topi, wmean = _host_router(x, gate_w, lb_bias)

    sel = [np.nonzero((topi == e).any(axis=-1))[0] for e in range(E)]
    counts = [len(s) for s in sel]
    C = max(_round_up(max(counts), 16), 128)

    nc = _get_nc(C)

    f16 = np.float16
    xfT = np.ascontiguousarray(xf.T)  # [H, T]
    xfT16 = xfT.astype(f16)

    # shared weights per SI-half (lhsT layouts)
    sgT_h = [np.ascontiguousarray(sgw[h * SIH:(h + 1) * SIH].T)
             .astype(f16).reshape(KH, P, SIH) for h in range(2)]
    suT_h = [np.ascontiguousarray(suw[h * SIH:(h + 1) * SIH].T)
             .astype(f16).reshape(KH, P, SIH) for h in range(2)]
    sdT_h = [np.ascontiguousarray(sdw[:, h * SIH:(h + 1) * SIH].T)
             .astype(f16).reshape(KS, P, H) for h in range(2)]

    in_maps = []
    for e in range(E):
        xe = np.zeros((H, C), f16)
        if counts[e]:
            xe[:, :counts[e]] = xfT16[:, sel[e]]
        wgT = np.ascontiguousarray(egw[e].T).astype(f16).reshape(KH, P, I)
        wuT = np.ascontiguousarray(euw[e].T).astype(f16).reshape(KH, P, I)
        wdT = np.ascontiguousarray((edw[e] * wmean[e]).T).astype(f16) \
            .reshape(KI, P, H)
        tsl = e % 4    # token-slice index
        sh = e // 4    # SI half
        xs = np.ascontiguousarray(
            xfT16[:, tsl * TS:(tsl + 1) * TS]).reshape(KH, P, TS)
        in_maps.append({
            "xe": xe.reshape(KH, P, C), "wg": wgT, "wu": wuT, "wd": wdT,
            "xs": xs, "sg": sgT_h[sh], "su": suT_h[sh], "sd": sdT_h[sh],
        })

    res = run_bass_kernel_spmd(nc, in_maps, core_ids=list(range(N_CORES)))

    # ---- host combine ----
    out = np.zeros((T, H), np.float32)
    for e in range(E):
        if counts[e]:
            ye = np.asarray(res.results[e]["ye"], np.float32).reshape(H, C)
            out[sel[e]] += ye[:, :counts[e]].T
        zsout = np.asarray(res.results[e]["zs"], np.float32).reshape(H, TS)
        tsl = e % 4
        out[tsl * TS:(tsl + 1) * TS] += zsout.T
    return out.reshape(B, S, H).astype(x.dtype)
